# revision 15
# baseline (speedup 1.0000x reference)
"""MinLSTM fused kernel for Trainium2 (8 NeuronCores, SPMD).

Math: the reference applies cumlogsumexp over the sequence but only the LAST
timestep feeds the output head, so the scan collapses to a single logsumexp
reduction over sequence:

    log_h_last = log_f[S-1] + log(0.5 + sum_s exp(diff_s + log_g(h_s)))
    out = exp(log_h_last) @ w_out.T + b_out

with diff = softplus(-f) - softplus(-i) and per-token term

    exp(diff + log_g(h)) = (1 + e^{-f}) * sigmoid(i) * g(h)
                         = 1/4 * (1+e^{-f}) * (1+tanh(i/2)) * max(1+2h, 1+tanh(h/2))

which needs only {exp, tanh, copy} — all in the ACT `exp_and_others` table
(single table load). The device fuses the z = x @ w_in.T matmul (fp8
DoubleRow, fp32 PSUM accumulation) with the per-token nonlinearity and the
per-(batch, channel) partial sums. The host applies the exact last-token
correction in fp64 and runs the tiny [4,1024]x[1024,1024] output head.

Sharding: hidden-channel parallel — core c owns h-channels [c*128, (c+1)*128)
(i.e. 3 x 128 rows of w_in) and streams ALL 32768 tokens. This keeps the
per-core weight load to a single 384 KB DMA (vs streaming 3 MB of stripes),
makes every token block a full 512 (the moving-dim size where the PE is
stream-bound rather than LDWEIGHTS-bound), and leaves a single short
ACT->DVE drain after the last matmul. Per-core HBM traffic is 32 MB of fp8
x at ~195 GB/s sustained — well under the ~358 GB/s per-core ceiling.

Engine budget per 512-token block (PE period 2.59us):
  ACT: one 1024-elem tanh over the (i,h) 2-bank PSUM pair + copy(2h+1) +
       exp(-f)  ~2.1us
  DVE: 3 bf16 scalar_tensor_tensor + accumulator read  ~2.2us
PSUM reads stay on ACT only — a DVE op reading PSUM measurably saturates
the DVE and causes periodic PE write-after-read stalls.

Startup: weights and the first x block are DMA'd in kb-pair pieces on the
two HWDGE queues (w on scalar, x on sync); a dummy-matmul burst keeps the
PE busy so the HAM clock gate is open (2.4 GHz) right when the first pieces
land (~3.4us after body entry — DMA completion receipt and the HAM warmup
window happen to coincide).
"""

from contextlib import ExitStack

import ml_dtypes
import numpy as np

B, S, D, H = 4, 8192, 1024, 1024
N_CORES = 8
CH = H // N_CORES       # 128 h-channels per core
TOKS = B * S            # 32768 tokens, all streamed by every core
TB = 512                # token block (PSUM bank / moving free dim)
NTB = TOKS // TB        # 64
NBB = S // TB           # 16 blocks per batch sequence
KC = D // 128           # 8 contraction chunks of 128

USE_FP8 = True
WSCALE = 64.0           # w pre-scale so fp8 w values sit in the normal range
N_DUMMY = 52            # HAM pre-warm matmuls issued while startup DMAs fly

_CACHE = {}


def _build_nc(use_fp8):
    import concourse.bacc as bacc
    import concourse.mybir as mybir
    import concourse.tile as tile

    dt = mybir.dt
    AF = mybir.ActivationFunctionType
    ALU = mybir.AluOpType

    in_dt = dt.float8e4 if use_fp8 else dt.bfloat16
    inv = 1.0 / WSCALE if use_fp8 else 1.0

    nc = bacc.Bacc("TRN2", target_bir_lowering=False)
    # xt[tb, p, kc, s] = x_flat[tb*TB + s, kc*128 + p] — per-partition rows
    # are KC*TB contiguous bytes so each block DMA is dense 4 KB descriptors.
    xt = nc.dram_tensor("xt", (NTB, 128, KC, TB), in_dt, kind="ExternalInput")
    # wt[p, kc, g*128+c] = w_in[g*H + core*128 + c, kc*128 + p]
    wt = nc.dram_tensor("wt", (128, KC, 384), in_dt, kind="ExternalInput")
    # split outputs so only the final seq-block's cells ride the post-stream
    # tail, and both DMAs are fully contiguous (a strided slice DMA degrades
    # to 4-byte descriptors with ~5us completion latency)
    out_a = nc.dram_tensor("sums_a", (128, B, NBB - 1), dt.float32, kind="ExternalOutput")
    out_b = nc.dram_tensor("sums_b", (128, B), dt.float32, kind="ExternalOutput")
    out_c = nc.dram_tensor("sums_c", (128, 1), dt.float32, kind="ExternalOutput")

    with tile.TileContext(nc) as tc, ExitStack() as ctx:
        wpool = ctx.enter_context(tc.tile_pool(name="w", bufs=1))
        # x prefetch depth: the block-k+N DMA can only ISSUE once block k's
        # matmuls retire (write-after-read) and then needs ~2.2us to land
        # (HWDGE issue + transfer + completion receipt). bufs=3 leaves zero
        # slack — one hiccup phase-locks the whole stream into periodic
        # PE-waits-for-x stalls. bufs=5 gives ~3 blocks (~7.8us) of slack.
        xpool = ctx.enter_context(tc.tile_pool(name="x", bufs=6))
        gpool = ctx.enter_context(tc.tile_pool(name="g", bufs=2))
        spool = ctx.enter_context(tc.tile_pool(name="s", bufs=1))
        psih = ctx.enter_context(tc.tile_pool(name="psih", bufs=2, space="PSUM"))
        psf = ctx.enter_context(tc.tile_pool(name="psf", bufs=2, space="PSUM"))

        slab_a = spool.tile([128, B, NBB - 1], dt.float32)
        slab_b = spool.tile([128, B], dt.float32)
        slab_c = spool.tile([128, 1], dt.float32)

        # HAM pre-warm: the PE clock gate defaults to 1.2 GHz and opens after
        # ~3.4us of sustained activity; the startup DMAs need about that long
        # to land. Keep the PE busy so the real stream starts warm.
        dum = gpool.tile([128, 64], dt.bfloat16, tag="dum")
        nc.vector.memset(dum[:], 0.0)
        psd = psih.tile([128, 2, TB], dt.float32, tag="ps", bufs=2)
        for _ in range(N_DUMMY):
            nc.tensor.matmul(psd[0:64, 0, 0:64], dum[:], dum[:], start=True, stop=True)

        # Startup-critical DMAs: two halves each on the two HWDGE queues
        # (w on scalar, x0 on sync). Two pieces, not eight: each DMA issue
        # op costs ~0.65us of queue time, so fine slicing serializes on
        # issue; halves land at ~10.5/~11.4us which the first (cold, HAM
        # still warming) matmuls cannot outrun.
        w_all = wpool.tile([128, KC, 384], in_dt)
        x0 = xpool.tile([128, KC, TB], in_dt, tag="x")
        for hh in range(2):
            kl, kr = hh * (KC // 2), (hh + 1) * (KC // 2)
            nc.scalar.dma_start(w_all[:, kl:kr, :], wt[:, kl:kr, :])
            nc.sync.dma_start(x0[:, kl:kr, :], xt[0, :, kl:kr, :])

        # block schedule: 63 full 512-token blocks, then the final 512 split
        # 384+128 so the post-last-matmul ACT/DVE drain (which the scheduler's
        # own gate ordering controls) runs on a 128-token block — every drain
        # op shrinks 4x. The 128-block pays a small LDWEIGHTS-bound premium
        # (~0.3us) but removes ~2.5us from the serial tail.
        blocks = [(k * TB, TB) for k in range(NTB - 1)] + [
            ((NTB - 1) * TB, 384),
            ((NTB - 1) * TB + 384, 128),
        ]
        for nb, (toff, tsz) in enumerate(blocks):
            bb, ib = divmod(toff // TB, NBB)
            tb = toff // TB
            # accumulator cell: batch-b blocks 0..14 -> slab_a, the ib==15
            # full blocks (b<3) -> slab_b[:, b], the split pair -> slab_c
            # then slab_b[:, 3] (the only cell after the last matmul)
            if ib < NBB - 1:
                acc = slab_a[:, bb, ib : ib + 1]
            elif bb < B - 1:
                acc = slab_b[:, bb : bb + 1]
            elif tsz == 384:
                acc = slab_c[:, 0:1]
            else:
                acc = slab_b[:, B - 1 : B]
            if nb == 0:
                x_sb = x0
            elif tsz == TB:
                x_sb = xpool.tile([128, KC, TB], in_dt, tag="x")
                nc.sync.dma_start(x_sb[:], xt[tb])
            else:
                soff = toff - tb * TB
                x_sb = xpool.tile([128, KC, tsz], in_dt, tag=f"x{tsz}")
                nc.sync.dma_start(x_sb[:], xt[tb, :, :, soff : soff + tsz])

            pih_f = psih.tile([128, 2, TB], dt.float32, tag="ps", bufs=2)
            pf_f = psf.tile([128, TB], dt.float32, tag="ps", bufs=2)
            pih = pih_f[:, :, :tsz]
            pf = pf_f[:, :tsz]

            def mm(out_ap, g, kb, start, stop):
                nc.tensor.matmul(
                    out_ap,
                    w_all[:, 2 * kb : 2 * kb + 2, g * 128 : (g + 1) * 128],
                    x_sb[:, 2 * kb : 2 * kb + 2, :],
                    start=start,
                    stop=stop,
                    perf_mode=mybir.MatmulPerfMode.DoubleRow,
                    skip_group_check=(nb == 0),
                )

            # gate -> (psum target, weight column group): f=0, i=1, h=2
            tgt = {"h": (pih[:, 1, :], 2), "i": (pih[:, 0, :], 1), "f": (pf[:], 0)}
            if nb == 0:
                # interleave gates kb-major so each arriving startup DMA piece
                # feeds 3 matmuls while the pieces land
                for kb in range(KC // 2):
                    for gate in ("h", "i", "f"):
                        ap, g = tgt[gate]
                        mm(ap, g, kb, kb == 0, kb == KC // 2 - 1)
            else:
                for gate in ("h", "i", "f"):
                    ap, g = tgt[gate]
                    for kb in range(KC // 2):
                        mm(ap, g, kb, kb == 0, kb == KC // 2 - 1)

            h2_f = gpool.tile([128, TB], dt.bfloat16, tag="h2")
            a_f = gpool.tile([128, TB], dt.bfloat16, tag="a")
            h2, a = h2_f[:, :tsz], a_f[:, :tsz]
            nc.scalar.activation(h2[:], pih[:, 1, :], AF.Copy, scale=2.0 * inv, bias=1.0)
            tith_f = gpool.tile([128, 2, TB], dt.bfloat16, tag="tith")
            tith = tith_f[:, :, :tsz]
            nc.scalar.activation(tith[:], pih[:], AF.Tanh, scale=0.5 * inv)
            nc.scalar.activation(a[:], pf[:], AF.Exp, scale=-inv)
            t_i, t_h = tith[:, 0], tith[:, 1]

            # w2 = max(1+tanh(h/2), 1+2h);  r = (1+e^{-f}) * w2
            # t = (1+tanh(i/2)) * r, row-summed into the accumulator cell
            w2_f = gpool.tile([128, TB], dt.bfloat16, tag="w2")
            w2 = w2_f[:, :tsz]
            nc.vector.scalar_tensor_tensor(
                w2[:], t_h[:], 1.0, h2[:], op0=ALU.add, op1=ALU.max
            )
            r_f = gpool.tile([128, TB], dt.bfloat16, tag="r")
            r = r_f[:, :tsz]
            nc.vector.scalar_tensor_tensor(
                r[:], a[:], 1.0, w2[:], op0=ALU.add, op1=ALU.mult
            )
            t_f = gpool.tile([128, TB], dt.bfloat16, tag="t")
            t = t_f[:, :tsz]
            nc.vector.scalar_tensor_tensor(
                t[:],
                t_i[:],
                1.0,
                r[:],
                op0=ALU.add,
                op1=ALU.mult,
                accum_out=acc,
            )

            if nb == NTB - 2:
                # all slab_a cells are written once block 62 retires — ship
                # them while the final blocks still stream
                nc.sync.dma_start(out_a[:], slab_a[:])
            if nb == NTB - 1:
                # the 384-token half's cell; hides under the final 128-block
                nc.sync.dma_start(out_c[:], slab_c[:])

        nc.sync.dma_start(out_b[:], slab_b[:])

    nc.compile()
    return nc


def _get_nc():
    key = "fp8" if USE_FP8 else "bf16"
    if key not in _CACHE:
        _CACHE[key] = _build_nc(USE_FP8)
    return _CACHE[key]


def _softplus(v):
    return np.log1p(np.exp(-np.abs(v))) + np.maximum(v, 0.0)


def kernel(x, w_in, w_out, b_out, _return_results=False, _trace=False):
    from concourse.bass_utils import run_bass_kernel_spmd

    x = np.asarray(x)
    w_in = np.asarray(w_in)
    w_out = np.asarray(w_out)
    b_out = np.asarray(b_out)

    if USE_FP8:
        cast_dt = ml_dtypes.float8_e4m3  # TRN FP8_EXP4: max ±240, inf above

        def cast(a):
            return np.clip(a, -240.0, 240.0).astype(cast_dt)

        w_scaled = w_in * WSCALE
    else:
        cast_dt = ml_dtypes.bfloat16

        def cast(a):
            return a.astype(cast_dt)

        w_scaled = w_in

    # per-core weight pack: wt[p, kc, g*128+c] = w_scaled[g*H + core*128+c, kc*128+p]
    w5 = w_scaled.reshape(3, N_CORES, CH, KC, 128)
    wts = []
    for c in range(N_CORES):
        wc = np.ascontiguousarray(w5[:, c].transpose(3, 2, 0, 1))  # [128p, KC, 3, CH]
        wts.append(np.asarray(cast(wc)).reshape(128, KC, 384))

    # shared token pack: xt[tb, p, kc, s] = x_flat[tb*TB + s, kc*128 + p]
    xq = cast(x.reshape(TOKS, D))
    xt = np.ascontiguousarray(
        np.asarray(xq).reshape(NTB, TB, KC, 128).transpose(0, 3, 2, 1)
    )

    in_maps = [{"xt": xt, "wt": wts[c]} for c in range(N_CORES)]

    nc = _get_nc()
    # the first execution of a freshly compiled NEFF occasionally hits a
    # transient NRT exec error on this setup — retry once
    try:
        res = run_bass_kernel_spmd(
            nc, in_maps, core_ids=list(range(N_CORES)), trace=_trace
        )
    except Exception:
        import time as _time

        _time.sleep(2.0)
        res = run_bass_kernel_spmd(
            nc, in_maps, core_ids=list(range(N_CORES)), trace=False
        )

    # per core -> channel h = core*128 + p
    parts = []
    for r in res.results:
        s = np.asarray(r["sums_a"]).astype(np.float64).sum(axis=2) + np.asarray(
            r["sums_b"]
        ).astype(np.float64)
        s[:, B - 1] += np.asarray(r["sums_c"]).astype(np.float64)[:, 0]
        parts.append(s.T)
    Ssum = np.concatenate(parts, axis=1) * 0.25  # [B, H]

    # exact last-token factor in fp64 (host): log_f[S-1] = -softplus(diff[S-1])
    z_last = x[:, -1, :].astype(np.float64) @ w_in.astype(np.float64).T
    f_l, i_l = z_last[:, :H], z_last[:, H : 2 * H]
    diff_l = _softplus(-f_l) - _softplus(-i_l)
    h_last = np.exp(-_softplus(diff_l) + np.log(0.5 + Ssum))
    out = (h_last @ w_out.astype(np.float64).T + b_out.astype(np.float64)).astype(
        np.float32
    )
    if _return_results:
        return out, res
    return out


# revision 17
# speedup vs baseline: 1.0378x; 1.0378x over previous
"""MinLSTM fused kernel for Trainium2 (8 NeuronCores, SPMD).

Math: the reference applies cumlogsumexp over the sequence but only the LAST
timestep feeds the output head, so the scan collapses to a single logsumexp
reduction over sequence:

    log_h_last = log_f[S-1] + log(0.5 + sum_s exp(diff_s + log_g(h_s)))
    out = exp(log_h_last) @ w_out.T + b_out

with diff = softplus(-f) - softplus(-i) and per-token term

    exp(diff + log_g(h)) = (1 + e^{-f}) * sigmoid(i) * g(h)
                         = 1/4 * (1+e^{-f}) * (1+tanh(i/2)) * max(1+2h, 1+tanh(h/2))

which needs only {exp, tanh, copy} — all in the ACT `exp_and_others` table
(single table load). The device fuses the z = x @ w_in.T matmul (fp8
DoubleRow, fp32 PSUM accumulation) with the per-token nonlinearity and the
per-(batch, channel) partial sums. The host applies the exact last-token
correction in fp64 and runs the tiny [4,1024]x[1024,1024] output head.

Sharding: hidden-channel parallel — core c owns h-channels [c*128, (c+1)*128)
(i.e. 3 x 128 rows of w_in) and streams ALL 32768 tokens. This keeps the
per-core weight load to a single 384 KB DMA (vs streaming 3 MB of stripes),
makes every token block a full 512 (the moving-dim size where the PE is
stream-bound rather than LDWEIGHTS-bound), and leaves a single short
ACT->DVE drain after the last matmul. Per-core HBM traffic is 32 MB of fp8
x at ~195 GB/s sustained — well under the ~358 GB/s per-core ceiling.

Engine budget per 512-token block (PE period 2.59us = 12 DoubleRow matmuls):
  ACT: one 1024-elem tanh over the (i,h) 2-bank PSUM pair + copy(2h+1) +
       exp(-f)  ~2.1us
  DVE: 3 bf16 scalar_tensor_tensor + accumulator read  ~2.2us
PSUM reads stay on ACT only — a DVE op reading PSUM measurably saturates
the DVE and causes periodic PE write-after-read stalls.

Hard-won scheduling facts baked in below:
 - x prefetch needs bufs>=5: the block-k+N DMA can only issue after block
   k's matmuls retire and then takes ~2.2us to land; with bufs=3 one
   hiccup phase-locks the stream into periodic PE-waits-for-x stalls.
 - The Tile scheduler picks its own PSUM-group order per block (emission
   order of the three gate groups is NOT preserved), and readers of a
   2-bank pair tile wait for every group writing either bank. So the
   post-last-matmul ACT->DVE drain length is controlled by shrinking the
   final blocks (512 -> 384+128), not by op reordering.
 - Any DMA under ~16 bytes/partition (and any strided slice) degrades to
   tiny descriptors with a multi-us completion receipt; outputs are
   padded/contiguous, and everything except the final halves' cells ships
   one block early so only one small receipt rides the tail.

Startup: weights and the first x block are DMA'd in halves on the two
HWDGE queues (w on scalar, x on sync); a dummy-matmul burst keeps the PE
busy so the HAM clock gate is open (2.4 GHz) right when the first pieces
land (~3.4us after body entry — DMA completion receipt and the HAM warmup
window happen to coincide).
"""

from contextlib import ExitStack

import ml_dtypes
import numpy as np

B, S, D, H = 4, 8192, 1024, 1024
N_CORES = 8
CH = H // N_CORES       # 128 h-channels per core
TOKS = B * S            # 32768 tokens, all streamed by every core
TB = 512                # token block (PSUM bank / moving free dim)
NTB = TOKS // TB        # 64
NBB = S // TB           # 16 blocks per batch sequence
KC = D // 128           # 8 contraction chunks of 128

USE_FP8 = True
WSCALE = 64.0           # w pre-scale so fp8 w values sit in the normal range
N_DUMMY = 52            # HAM pre-warm matmuls issued while startup DMAs fly

_CACHE = {}


def _build_nc(use_fp8):
    import concourse.bacc as bacc
    import concourse.mybir as mybir
    import concourse.tile as tile

    dt = mybir.dt
    AF = mybir.ActivationFunctionType
    ALU = mybir.AluOpType

    in_dt = dt.float8e4 if use_fp8 else dt.bfloat16
    inv = 1.0 / WSCALE if use_fp8 else 1.0

    nc = bacc.Bacc("TRN2", target_bir_lowering=False)
    # xt[tb, p, kc, s] = x_flat[tb*TB + s, kc*128 + p] — per-partition rows
    # are KC*TB contiguous bytes so each block DMA is dense 4 KB descriptors.
    xt = nc.dram_tensor("xt", (NTB, 128, KC, TB), in_dt, kind="ExternalInput")
    # wt[p, kc, g*128+c] = w_in[g*H + core*128 + c, kc*128 + p]
    wt = nc.dram_tensor("wt", (128, KC, 384), in_dt, kind="ExternalInput")
    # split outputs so only the final seq-block's cells ride the post-stream
    # tail, and both DMAs are fully contiguous (a strided slice DMA degrades
    # to 4-byte descriptors with ~5us completion latency)
    out_a = nc.dram_tensor("sums_a", (128, B, NBB - 1), dt.float32, kind="ExternalOutput")
    # cells 0..2: per-batch ib==15 full blocks; 3: the 384 half; 4: the final
    # 128 half; 5..7 padding (a DMA under ~16 bytes/partition degrades to
    # tiny descriptors with multi-us completion receipt)
    out_b = nc.dram_tensor("sums_b", (128, 8), dt.float32, kind="ExternalOutput")

    with tile.TileContext(nc) as tc, ExitStack() as ctx:
        wpool = ctx.enter_context(tc.tile_pool(name="w", bufs=1))
        # x prefetch depth: the block-k+N DMA can only ISSUE once block k's
        # matmuls retire (write-after-read) and then needs ~2.2us to land
        # (HWDGE issue + transfer + completion receipt). bufs=3 leaves zero
        # slack — one hiccup phase-locks the whole stream into periodic
        # PE-waits-for-x stalls. bufs=5 gives ~3 blocks (~7.8us) of slack.
        xpool = ctx.enter_context(tc.tile_pool(name="x", bufs=6))
        gpool = ctx.enter_context(tc.tile_pool(name="g", bufs=2))
        spool = ctx.enter_context(tc.tile_pool(name="s", bufs=1))
        psih = ctx.enter_context(tc.tile_pool(name="psih", bufs=2, space="PSUM"))
        psf = ctx.enter_context(tc.tile_pool(name="psf", bufs=2, space="PSUM"))

        slab_a = spool.tile([128, B, NBB - 1], dt.float32)
        slab_b = spool.tile([128, 8], dt.float32)

        # HAM pre-warm: the PE clock gate defaults to 1.2 GHz and opens after
        # ~3.4us of sustained activity; the startup DMAs need about that long
        # to land. Keep the PE busy so the real stream starts warm.
        dum = gpool.tile([128, 64], dt.bfloat16, tag="dum")
        nc.vector.memset(dum[:], 0.0)
        psd = psih.tile([128, 2, TB], dt.float32, tag="ps", bufs=2)
        for _ in range(N_DUMMY):
            nc.tensor.matmul(psd[0:64, 0, 0:64], dum[:], dum[:], start=True, stop=True)

        # Startup-critical DMAs: two halves each on the two HWDGE queues
        # (w on scalar, x0 on sync). Two pieces, not eight: each DMA issue
        # op costs ~0.65us of queue time, so fine slicing serializes on
        # issue; halves land at ~10.5/~11.4us which the first (cold, HAM
        # still warming) matmuls cannot outrun.
        w_all = wpool.tile([128, KC, 384], in_dt)
        x0 = xpool.tile([128, KC, TB], in_dt, tag="x")
        for hh in range(2):
            kl, kr = hh * (KC // 2), (hh + 1) * (KC // 2)
            nc.scalar.dma_start(w_all[:, kl:kr, :], wt[:, kl:kr, :])
            nc.sync.dma_start(x0[:, kl:kr, :], xt[0, :, kl:kr, :])

        # block schedule: 63 full 512-token blocks, then the final 512 split
        # 384+128 so the post-last-matmul ACT/DVE drain (which the scheduler's
        # own gate ordering controls) runs on a 128-token block — every drain
        # op shrinks 4x. The 128-block pays a small LDWEIGHTS-bound premium
        # (~0.3us) but removes ~2.5us from the serial tail.
        blocks = [(k * TB, TB) for k in range(NTB - 1)] + [
            ((NTB - 1) * TB, 384),
            ((NTB - 1) * TB + 384, 128),
        ]
        for nb, (toff, tsz) in enumerate(blocks):
            bb, ib = divmod(toff // TB, NBB)
            tb = toff // TB
            # accumulator cell: batch-b blocks 0..14 -> slab_a, the ib==15
            # full blocks (b<3) -> slab_b[:, b], the split pair -> slab_c
            # then slab_b[:, 3] (the only cell after the last matmul)
            if ib < NBB - 1:
                acc = slab_a[:, bb, ib : ib + 1]
            elif bb < B - 1:
                acc = slab_b[:, bb : bb + 1]
            elif tsz == 384:
                acc = slab_b[:, 3:4]
            else:
                acc = slab_b[:, 4:5]
            if nb == 0:
                x_sb = x0
            elif tsz == TB:
                x_sb = xpool.tile([128, KC, TB], in_dt, tag="x")
                nc.sync.dma_start(x_sb[:], xt[tb])
            else:
                soff = toff - tb * TB
                x_sb = xpool.tile([128, KC, tsz], in_dt, tag=f"x{tsz}")
                nc.sync.dma_start(x_sb[:], xt[tb, :, :, soff : soff + tsz])

            pih_f = psih.tile([128, 2, TB], dt.float32, tag="ps", bufs=2)
            pf_f = psf.tile([128, TB], dt.float32, tag="ps", bufs=2)
            pih = pih_f[:, :, :tsz]
            pf = pf_f[:, :tsz]

            def mm(out_ap, g, kb, start, stop):
                nc.tensor.matmul(
                    out_ap,
                    w_all[:, 2 * kb : 2 * kb + 2, g * 128 : (g + 1) * 128],
                    x_sb[:, 2 * kb : 2 * kb + 2, :],
                    start=start,
                    stop=stop,
                    perf_mode=mybir.MatmulPerfMode.DoubleRow,
                    skip_group_check=(nb == 0),
                )

            # gate -> (psum target, weight column group): f=0, i=1, h=2
            tgt = {"h": (pih[:, 1, :], 2), "i": (pih[:, 0, :], 1), "f": (pf[:], 0)}
            if nb == 0:
                # interleave gates kb-major so each arriving startup DMA piece
                # feeds 3 matmuls while the pieces land
                for kb in range(KC // 2):
                    for gate in ("h", "i", "f"):
                        ap, g = tgt[gate]
                        mm(ap, g, kb, kb == 0, kb == KC // 2 - 1)
            else:
                for gate in ("h", "i", "f"):
                    ap, g = tgt[gate]
                    for kb in range(KC // 2):
                        mm(ap, g, kb, kb == 0, kb == KC // 2 - 1)

            h2_f = gpool.tile([128, TB], dt.bfloat16, tag="h2")
            a_f = gpool.tile([128, TB], dt.bfloat16, tag="a")
            h2, a = h2_f[:, :tsz], a_f[:, :tsz]
            nc.scalar.activation(h2[:], pih[:, 1, :], AF.Copy, scale=2.0 * inv, bias=1.0)
            tith_f = gpool.tile([128, 2, TB], dt.bfloat16, tag="tith")
            tith = tith_f[:, :, :tsz]
            nc.scalar.activation(tith[:], pih[:], AF.Tanh, scale=0.5 * inv)
            nc.scalar.activation(a[:], pf[:], AF.Exp, scale=-inv)
            t_i, t_h = tith[:, 0], tith[:, 1]

            # w2 = max(1+tanh(h/2), 1+2h);  r = (1+e^{-f}) * w2
            # t = (1+tanh(i/2)) * r, row-summed into the accumulator cell
            w2_f = gpool.tile([128, TB], dt.bfloat16, tag="w2")
            w2 = w2_f[:, :tsz]
            nc.vector.scalar_tensor_tensor(
                w2[:], t_h[:], 1.0, h2[:], op0=ALU.add, op1=ALU.max
            )
            r_f = gpool.tile([128, TB], dt.bfloat16, tag="r")
            r = r_f[:, :tsz]
            nc.vector.scalar_tensor_tensor(
                r[:], a[:], 1.0, w2[:], op0=ALU.add, op1=ALU.mult
            )
            t_f = gpool.tile([128, TB], dt.bfloat16, tag="t")
            t = t_f[:, :tsz]
            nc.vector.scalar_tensor_tensor(
                t[:],
                t_i[:],
                1.0,
                r[:],
                op0=ALU.add,
                op1=ALU.mult,
                accum_out=acc,
            )

            if nb == NTB - 2:
                # all slab_a cells are written once block 62 retires — ship
                # them while the final blocks still stream
                nc.sync.dma_start(out_a[:], slab_a[:])
        nc.sync.dma_start(out_b[:], slab_b[:])

    nc.compile()
    return nc


def _get_nc():
    key = "fp8" if USE_FP8 else "bf16"
    if key not in _CACHE:
        _CACHE[key] = _build_nc(USE_FP8)
    return _CACHE[key]


def _softplus(v):
    return np.log1p(np.exp(-np.abs(v))) + np.maximum(v, 0.0)


def kernel(x, w_in, w_out, b_out, _return_results=False, _trace=False):
    from concourse.bass_utils import run_bass_kernel_spmd

    x = np.asarray(x)
    w_in = np.asarray(w_in)
    w_out = np.asarray(w_out)
    b_out = np.asarray(b_out)

    if USE_FP8:
        cast_dt = ml_dtypes.float8_e4m3  # TRN FP8_EXP4: max ±240, inf above

        def cast(a):
            return np.clip(a, -240.0, 240.0).astype(cast_dt)

        w_scaled = w_in * WSCALE
    else:
        cast_dt = ml_dtypes.bfloat16

        def cast(a):
            return a.astype(cast_dt)

        w_scaled = w_in

    # per-core weight pack: wt[p, kc, g*128+c] = w_scaled[g*H + core*128+c, kc*128+p]
    w5 = w_scaled.reshape(3, N_CORES, CH, KC, 128)
    wts = []
    for c in range(N_CORES):
        wc = np.ascontiguousarray(w5[:, c].transpose(3, 2, 0, 1))  # [128p, KC, 3, CH]
        wts.append(np.asarray(cast(wc)).reshape(128, KC, 384))

    # shared token pack: xt[tb, p, kc, s] = x_flat[tb*TB + s, kc*128 + p]
    xq = cast(x.reshape(TOKS, D))
    xt = np.ascontiguousarray(
        np.asarray(xq).reshape(NTB, TB, KC, 128).transpose(0, 3, 2, 1)
    )

    in_maps = [{"xt": xt, "wt": wts[c]} for c in range(N_CORES)]

    nc = _get_nc()
    # the first execution of a freshly compiled NEFF occasionally hits a
    # transient NRT exec error on this setup — retry once
    try:
        res = run_bass_kernel_spmd(
            nc, in_maps, core_ids=list(range(N_CORES)), trace=_trace
        )
    except Exception:
        import time as _time

        _time.sleep(2.0)
        res = run_bass_kernel_spmd(
            nc, in_maps, core_ids=list(range(N_CORES)), trace=False
        )

    # per core -> channel h = core*128 + p
    parts = []
    for r in res.results:
        sb = np.asarray(r["sums_b"]).astype(np.float64)
        s = np.asarray(r["sums_a"]).astype(np.float64).sum(axis=2)
        s[:, :3] += sb[:, :3]
        s[:, B - 1] += sb[:, 3] + sb[:, 4]
        parts.append(s.T)
    Ssum = np.concatenate(parts, axis=1) * 0.25  # [B, H]

    # exact last-token factor in fp64 (host): log_f[S-1] = -softplus(diff[S-1])
    z_last = x[:, -1, :].astype(np.float64) @ w_in.astype(np.float64).T
    f_l, i_l = z_last[:, :H], z_last[:, H : 2 * H]
    diff_l = _softplus(-f_l) - _softplus(-i_l)
    h_last = np.exp(-_softplus(diff_l) + np.log(0.5 + Ssum))
    out = (h_last @ w_out.astype(np.float64).T + b_out.astype(np.float64)).astype(
        np.float32
    )
    if _return_results:
        return out, res
    return out
